# revision 12
# baseline (speedup 1.0000x reference)
"""MultiHeadLocalAttention Trainium2 kernel.

Strategy: shard the sequence across 8 NeuronCores (1024 q-tokens per core per
batch) with a 128-token KV halo on each side (handled host-side by overlapped
slicing + zero padding).  Everything else is token-local, so no collectives.

Per-core Bass/Tile program (fp16 on-chip storage, fp32 PSUM accumulation):
  QT = Wq'^T x^T, KT = Wk^T x^T    ([head*dim, tok] layout, Wq pre-scaled 1/8)
  V  = x Wv                        ([tok, head*dim] layout, +ones column)
  per (batch, 128-q-block, head-group):  sT[j,(par,q)] = KT_blk^T @ QT_blk
  (transposed scores, 3 key blocks, both head parities in one matmul via the
  zero-padded par slots of qTz), e = exp(sT) (no max subtraction: |s| < ~2 by
  construction), triangular masks multiply the two side blocks,
  O[q,d+1] = sum_j e_j^T @ [V_j | 1]  (fused denominator), O *= 1/denom,
  OT via identity matmul, out = OT^T @ Wo.

Pipelining: PSUM = scores [P,6,256]x2bufs (6 banks) + AV [P,4,128] (1 bank)
+ big [P,512] (1 bank); projections alternate the av/big banks for
double-buffered psum->sbuf copies, which are round-robined across the
Act/DVE/Pool engines.  Output is written fp16 and widened host-side.
"""

import os
import sys

import numpy as np

if "/opt/trn_rl_repo" not in sys.path:
    sys.path.insert(0, "/opt/trn_rl_repo")

_STAGE = os.environ.get("K_STAGE", "full")  # proj | noav | notail | full

B, S, F = 2, 8192, 512
H, D = 8, 64
BLK = 128
NCORES = 8
T = S // NCORES           # 1024 q tokens per core per batch
NQ = T // BLK             # 8 q blocks
TKV = T + 2 * BLK         # 1280 kv tokens incl halo
NKV = TKV // BLK          # 10 kv blocks
P = 128
import ml_dtypes
F16 = ml_dtypes.bfloat16


def _build_program(with_qk_bias, with_v_bias, with_o_bias):
    import concourse.bass as bass
    import concourse.bacc as bacc
    import concourse.mybir as mybir
    import concourse.tile as tile

    f16 = mybir.dt.bfloat16
    f32 = mybir.dt.float32

    nc = bacc.Bacc("TRN2", target_bir_lowering=False, debug=False)

    xq_d = nc.dram_tensor("xqT", [P, 4, B, T], f16, kind="ExternalInput").ap()
    xkv_d = nc.dram_tensor("xkvT", [P, 4, B, TKV], f16, kind="ExternalInput").ap()
    wq_d = nc.dram_tensor("wq", [P, 4, 512], f16, kind="ExternalInput").ap()
    wk_d = nc.dram_tensor("wk", [P, 4, 512], f16, kind="ExternalInput").ap()
    wv_d = nc.dram_tensor("wv", [P, 4, 512], f16, kind="ExternalInput").ap()
    wo_d = nc.dram_tensor("wo", [P, 4, 512], f16, kind="ExternalInput").ap()
    masks_d = nc.dram_tensor("masks", [P, NQ, 2, BLK], f16, kind="ExternalInput").ap()
    ident_d = nc.dram_tensor("ident", [P, P], f16, kind="ExternalInput").ap()
    if with_qk_bias:
        bqk_d = nc.dram_tensor("bqk", [P, 8], f32, kind="ExternalInput").ap()
    if with_v_bias:
        bv_d = nc.dram_tensor("bv", [P, 512], f16, kind="ExternalInput").ap()
    if with_o_bias:
        bo_d = nc.dram_tensor("bo", [P, 512], f32, kind="ExternalInput").ap()
    out_d = nc.dram_tensor("out", [B, T, F], f16, kind="ExternalOutput").ap()

    Exp = mybir.ActivationFunctionType.Exp
    mult = mybir.AluOpType.mult

    with tile.TileContext(nc) as tc:
        with (
            tc.tile_pool(name="persist", bufs=1) as sb,
            tc.tile_pool(name="epool", bufs=3) as epool,
            tc.tile_pool(name="opool", bufs=2) as opool,
            tc.tile_pool(name="otpool", bufs=2) as otpool,
            tc.tile_pool(name="rpool", bufs=3) as rpool,
            tc.tile_pool(name="dpool", bufs=4) as dpool,
            tc.tile_pool(name="ps_s", bufs=2, space="PSUM") as ps_s_pool,
            tc.tile_pool(name="ps_av", bufs=1, space="PSUM") as ps_av_pool,
            tc.tile_pool(name="ps_big", bufs=1, space="PSUM") as ps_big_pool,
        ):
            # ---- persistent SBUF tensors ----
            wq_sb = sb.tile([P, 4, 512], f16, tag="wq")
            wk_sb = sb.tile([P, 4, 512], f16, tag="wk")
            wv_sb = sb.tile([P, 4, 512], f16, tag="wv")
            wo_sb = sb.tile([P, 4, 512], f16, tag="wo")
            masks_sb = sb.tile([P, NQ, 2, BLK], f16, tag="masks")
            id_sb = sb.tile([P, P], f16, tag="ident")
            xq_sb = sb.tile([P, 4, B, T], f16, tag="xq")
            xkv_sb = sb.tile([P, 4, B, TKV], f16, tag="xkv")
            # Q^T with explicit head-parity (par) slots: par 0 holds even heads
            # on partitions 0:64 (rest zero), par 1 holds odd heads on
            # partitions 64:128 (rest zero).  One score matmul then streams
            # both parities (N=256) against the shared kT block.  Matmuls with
            # base_partition=64 operands wedge the PE on this stack, hence the
            # zero-padding instead of 64-deep contractions.
            qTz_sb = sb.tile([P, 2, 4, B, T], f16, tag="qTz")
            kT_sb = sb.tile([P, 4, B, TKV], f16, tag="kT")
            v_sb = sb.tile([P, B, NKV, H * 65], f16, tag="v")

            # zero the pad halves before any DMA lands so they overlap the
            # input transfers (gpsimd memsets measured 3x slower: keep on DVE)
            nc.vector.memset(qTz_sb[64:128, 0], 0.0)
            nc.vector.memset(qTz_sb[0:64, 1], 0.0)
            # ones columns of V (col 64 of each head's 65-wide stripe)
            for h in range(H):
                nc.gpsimd.memset(v_sb[:, :, :, h * 65 + 64], 1.0)

            # ---- input DMAs, ordered by first use so compute starts early ----
            nc.sync.dma_start(xq_sb[:, :, 0], xq_d[:, :, 0])
            nc.sync.dma_start(wq_sb[:], wq_d[:])
            nc.sync.dma_start(xkv_sb[:, :, 0], xkv_d[:, :, 0])
            nc.sync.dma_start(wk_sb[:], wk_d[:])
            nc.sync.dma_start(wv_sb[:], wv_d[:])
            nc.sync.dma_start(xq_sb[:, :, 1], xq_d[:, :, 1])
            nc.sync.dma_start(xkv_sb[:, :, 1], xkv_d[:, :, 1])
            nc.sync.dma_start(wo_sb[:], wo_d[:])
            nc.sync.dma_start(masks_sb[:], masks_d[:])
            nc.sync.dma_start(id_sb[:], ident_d[:])
            if with_qk_bias:
                bqk_sb = sb.tile([P, 8], f32, tag="bqk")
                nc.sync.dma_start(bqk_sb[:], bqk_d[:])
            if with_v_bias:
                bv_sb = sb.tile([P, 512], f16, tag="bv")
                nc.sync.dma_start(bv_sb[:], bv_d[:])
            if with_o_bias:
                bo_sb = sb.tile([P, 512], f32, tag="bo")
                nc.sync.dma_start(bo_sb[:], bo_d[:])

            # ---- projections ----
            # psum->sbuf copy engines round-robin; psum banks alternate av/big
            _eng = [0]
            _bank = [0]

            def proj_psum():
                _bank[0] ^= 1
                if _bank[0]:
                    return ps_big_pool.tile([P, 512], f32, tag="big", name="ps_pb")
                return ps_av_pool.tile([P, 512], f32, tag="av", name="ps_pa")

            def copy_rr(dst, src):
                e = _eng[0] = (_eng[0] + 1) % 2
                if e == 0:
                    nc.scalar.copy(dst, src)
                else:
                    nc.vector.tensor_copy(dst, src)

            # QT/KT: lhsT = W tile [f, dh], rhs = xT [f, tok]
            def proj_T(w_sb, x_sb, dsts, tok_total, bias_col, b):
                per_b = tok_total // B
                for dh_t in range(4):
                    for off in range(0, per_b, 512):
                        w = min(512, per_b - off)
                        ps = proj_psum()
                        for ft in range(4):
                            nc.tensor.matmul(
                                ps[:, :w],
                                lhsT=w_sb[:, ft, dh_t * P:(dh_t + 1) * P],
                                rhs=x_sb[:, ft, b, off:off + w],
                                start=(ft == 0),
                                stop=(ft == 3),
                            )
                        for dst_sb, lo, hi_ in dsts:
                            dst = dst_sb(dh_t, b, off, w, lo, hi_)
                            src = ps[lo:hi_, :w]
                            if bias_col is not None:
                                nc.vector.tensor_scalar_add(
                                    dst, src,
                                    bqk_sb[lo:hi_, bias_col + dh_t:bias_col + dh_t + 1],
                                )
                            else:
                                copy_rr(dst, src)

            def qtz_dst(par):
                def f(dh_t, b, off, w, lo, hi_):
                    return qTz_sb[lo:hi_, par, dh_t, b, off:off + w]
                return f

            def kt_dst(dh_t, b, off, w, lo, hi_):
                return kT_sb[lo:hi_, dh_t, b, off:off + w]

            for b in range(B):
                proj_T(wq_sb, xq_sb,
                       [(qtz_dst(0), 0, 64), (qtz_dst(1), 64, 128)],
                       B * T, 0 if with_qk_bias else None, b)
                proj_T(wk_sb, xkv_sb, [(kt_dst, 0, 128)],
                       B * TKV, 4 if with_qk_bias else None, b)
                # V[tok, dh]: lhsT = xT tile [f, tok], rhs = Wv [f, dh]
                for kt in range(NKV):
                    ps = proj_psum()
                    for ft in range(4):
                        nc.tensor.matmul(
                            ps[:],
                            lhsT=xkv_sb[:, ft, b, kt * P:(kt + 1) * P],
                            rhs=wv_sb[:, ft, :],
                            start=(ft == 0),
                            stop=(ft == 3),
                        )
                    v_dst = v_sb[:, b, kt, :].rearrange("p (h x) -> p h x", h=H)[:, :, :64]
                    ps_v = ps.rearrange("p (h x) -> p h x", x=64)
                    if with_v_bias:
                        nc.vector.tensor_tensor(
                            v_dst, ps_v,
                            bv_sb.rearrange("p (h x) -> p h x", x=64), mybir.AluOpType.add,
                        )
                    else:
                        copy_rr(v_dst, ps_v)

            # ---- attention ----
            for b in range(B):
                if _STAGE == "proj":
                    break
                for qt in range(NQ):
                    o_t = opool.tile([P, 512], f16, tag="o")
                    rec_t = dpool.tile([P, H], f32, tag="rec")
                    for hg in range(2):
                        # scores: one matmul per (pr, j) streams both parities
                        ps_sc = ps_s_pool.tile([P, 6, 256], f32, tag="sc")
                        for pr in range(2):
                            dt_i = hg * 2 + pr
                            for j in range(3):
                                nc.tensor.matmul(
                                    ps_sc[:, pr * 3 + j, :],
                                    lhsT=kT_sb[:, dt_i, b, (qt + j) * P:(qt + j + 1) * P],
                                    rhs=qTz_sb[:, :, dt_i, b, qt * P:(qt + 1) * P],
                                    start=True, stop=True,
                                )
                        e_t = epool.tile([P, 6, 256], f16, tag="e")
                        nc.scalar.activation(e_t[:], ps_sc[:], Exp)
                        # both side-block masks in one strided op per pr
                        # (j in {0,2}); e layout (pr, j, par, q)
                        e5 = e_t.rearrange("p (pr j) (par q) -> p pr j par q", pr=2, par=2)
                        for pr in range(2):
                            nc.vector.tensor_tensor(
                                e5[:, pr, 0::2], e5[:, pr, 0::2],
                                masks_sb[:, qt, :, None, :].to_broadcast((P, 2, 2, P)),
                                mult,
                            )
                        if _STAGE == "noav":
                            continue
                        ps_av = ps_av_pool.tile([P, 4, 128], f32, tag="av")
                        for pr in range(2):
                            for par in range(2):
                                hi = pr * 2 + par
                                h = hg * 4 + hi
                                for j in range(3):
                                    nc.tensor.matmul(
                                        ps_av[:, hi, :65],
                                        lhsT=e5[:, pr, j, par, :],
                                        rhs=v_sb[:, b, qt + j, h * 65:(h + 1) * 65],
                                        start=(j == 0), stop=(j == 2),
                                    )
                        hs = slice(hg * 4, hg * 4 + 4)
                        nc.vector.reciprocal(rec_t[:, hs], ps_av[:, :, 64])
                        nc.vector.tensor_tensor(
                            o_t[:, hg * 256:(hg + 1) * 256].rearrange("p (h x) -> p h x", x=64),
                            ps_av[:, :, 0:64],
                            rec_t[:, hs][:, :, None].to_broadcast((P, 4, 64)),
                            mult,
                        )
                    if _STAGE in ("noav", "notail"):
                        if _STAGE == "notail":
                            res_t = rpool.tile([P, 512], f16, tag="res")
                            nc.vector.tensor_copy(res_t[:], o_t[:])
                            nc.sync.dma_start(out_d[b, qt * P:(qt + 1) * P, :], res_t[:])
                        continue
                    # transpose O (regular matmuls against identity), then out proj
                    ps_ot = ps_big_pool.tile([P, 512], f32, tag="big")
                    for dt_i in range(4):
                        nc.tensor.matmul(
                            ps_ot[:, dt_i * P:(dt_i + 1) * P],
                            lhsT=o_t[:, dt_i * P:(dt_i + 1) * P],
                            rhs=id_sb[:],
                            start=True, stop=True,
                        )
                    ot_t = otpool.tile([P, 512], f16, tag="ot")
                    nc.scalar.copy(ot_t[:], ps_ot[:])
                    ps_r = ps_av_pool.tile([P, 512], f32, tag="av")
                    for dt_i in range(4):
                        nc.tensor.matmul(
                            ps_r[:],
                            lhsT=ot_t[:, dt_i * P:(dt_i + 1) * P],
                            rhs=wo_sb[:, dt_i, :],
                            start=(dt_i == 0), stop=(dt_i == 3),
                        )
                    res_t = rpool.tile([P, 512], f16, tag="res")
                    if with_o_bias:
                        nc.vector.tensor_tensor(res_t[:], ps_r[:], bo_sb[:], mybir.AluOpType.add)
                    else:
                        nc.vector.tensor_copy(res_t[:], ps_r[:])
                    nc.sync.dma_start(out_d[b, qt * P:(qt + 1) * P, :], res_t[:])

    nc.compile()
    return nc


def _part_major(a2d):
    """[K*128, N] -> [128, K, N] partition-major contiguous fp16."""
    k = a2d.shape[0] // P
    return np.ascontiguousarray(
        a2d.reshape(k, P, *a2d.shape[1:]).transpose(1, 0, *range(2, a2d.ndim + 1))
    )


def kernel(**inputs):
    inputs_q = np.asarray(inputs["inputs_q"], np.float32)
    inputs_kv = np.asarray(inputs["inputs_kv"], np.float32)
    Wq = np.asarray(inputs["Wq"], np.float32).reshape(F, H * D) * np.float32(1.0 / np.sqrt(D))
    Wk = np.asarray(inputs["Wk"], np.float32).reshape(F, H * D)
    Wv = np.asarray(inputs["Wv"], np.float32).reshape(F, H * D)
    Wo = np.asarray(inputs["Wo"], np.float32).reshape(H * D, F)
    bq = np.asarray(inputs["bq"], np.float32).reshape(H * D) * np.float32(1.0 / np.sqrt(D))
    bk = np.asarray(inputs["bk"], np.float32).reshape(H * D)
    bv = np.asarray(inputs["bv"], np.float32).reshape(H * D)
    bo = np.asarray(inputs["bo"], np.float32).reshape(F)

    with_qk_bias = bool(np.any(bq) or np.any(bk))
    with_v_bias = bool(np.any(bv))
    with_o_bias = bool(np.any(bo))

    nc = _build_program(with_qk_bias, with_v_bias, with_o_bias)

    wq_h = _part_major(Wq.astype(F16))
    wk_h = _part_major(Wk.astype(F16))
    wv_h = _part_major(Wv.astype(F16))
    wo_h = _part_major(Wo.astype(F16))
    ident = np.eye(P, dtype=F16)

    xq16 = inputs_q.astype(F16)
    xkv16 = inputs_kv.astype(F16)

    maskL = np.tril(np.ones((BLK, BLK), F16))
    maskR = np.triu(np.ones((BLK, BLK), F16), 1)
    zero = np.zeros((BLK, BLK), F16)

    in_maps = []
    for c in range(NCORES):
        t0 = c * T
        xq_c = xq16[:, t0:t0 + T, :]                      # [B, T, F]
        lo, hi = t0 - BLK, t0 + T + BLK
        kv_c = np.pad(
            xkv16[:, max(0, lo):min(S, hi), :],
            ((0, 0), (max(0, -lo), max(0, hi - S)), (0, 0)),
        )                                                  # [B, TKV, F]

        # x^T in [128, ft, b, t] layout
        xqT = np.ascontiguousarray(
            xq_c.transpose(2, 0, 1).reshape(4, P, B, T).transpose(1, 0, 2, 3)
        )
        xkvT = np.ascontiguousarray(
            kv_c.transpose(2, 0, 1).reshape(4, P, B, TKV).transpose(1, 0, 2, 3)
        )

        masks = np.empty((P, NQ, 2, BLK), F16)
        for n in range(NQ):
            g = c * NQ + n
            masks[:, n, 0] = maskL if g > 0 else zero
            masks[:, n, 1] = maskR if g < S // BLK - 1 else zero

        m = {
            "xqT": xqT, "xkvT": xkvT,
            "wq": wq_h, "wk": wk_h, "wv": wv_h, "wo": wo_h,
            "masks": masks, "ident": ident,
        }
        if with_qk_bias:
            m["bqk"] = np.ascontiguousarray(
                np.stack([bq.reshape(4, P).T, bk.reshape(4, P).T], 1).reshape(P, 8)
            )
        if with_v_bias:
            m["bv"] = np.broadcast_to(bv.astype(F16), (P, 512)).copy()
        if with_o_bias:
            m["bo"] = np.broadcast_to(bo.astype(np.float32), (P, 512)).copy()
        in_maps.append(m)

    from concourse.bass_utils import run_bass_kernel_spmd

    res = run_bass_kernel_spmd(nc, in_maps, core_ids=list(range(NCORES)))
    global LAST_RESULT
    LAST_RESULT = res
    out = np.concatenate([np.asarray(res.results[c]["out"]) for c in range(NCORES)], axis=1)
    return np.ascontiguousarray(out.astype(np.float32))


LAST_RESULT = None


# revision 13
# speedup vs baseline: 1.0094x; 1.0094x over previous
"""MultiHeadLocalAttention Trainium2 kernel.

Strategy: shard the sequence across 8 NeuronCores (1024 q-tokens per core per
batch) with a 128-token KV halo on each side (handled host-side by overlapped
slicing + zero padding).  Everything else is token-local, so no collectives.

Per-core Bass/Tile program (fp16 on-chip storage, fp32 PSUM accumulation):
  QT = Wq'^T x^T, KT = Wk^T x^T    ([head*dim, tok] layout, Wq pre-scaled 1/8)
  V  = x Wv                        ([tok, head*dim] layout, +ones column)
  per (batch, 128-q-block, head-group):  sT[j,(par,q)] = KT_blk^T @ QT_blk
  (transposed scores, 3 key blocks, both head parities in one matmul via the
  zero-padded par slots of qTz), e = exp(sT) (no max subtraction: |s| < ~2 by
  construction), triangular masks multiply the two side blocks,
  O[q,d+1] = sum_j e_j^T @ [V_j | 1]  (fused denominator), O *= 1/denom,
  OT via identity matmul, out = OT^T @ Wo.

Pipelining: PSUM = scores [P,6,256]x2bufs (6 banks) + AV [P,4,128] (1 bank)
+ big [P,512] (1 bank); projections alternate the av/big banks for
double-buffered psum->sbuf copies, which are round-robined across the
Act/DVE/Pool engines.  Output is written fp16 and widened host-side.
"""

import os
import sys

import numpy as np

if "/opt/trn_rl_repo" not in sys.path:
    sys.path.insert(0, "/opt/trn_rl_repo")

_STAGE = os.environ.get("K_STAGE", "full")  # proj | noav | notail | full

B, S, F = 2, 8192, 512
H, D = 8, 64
BLK = 128
NCORES = 8
T = S // NCORES           # 1024 q tokens per core per batch
NQ = T // BLK             # 8 q blocks
TKV = T + 2 * BLK         # 1280 kv tokens incl halo
NKV = TKV // BLK          # 10 kv blocks
P = 128
import ml_dtypes
F16 = np.float16
F8 = ml_dtypes.float8_e3m4


def _build_program(with_qk_bias, with_v_bias, with_o_bias):
    import concourse.bass as bass
    import concourse.bacc as bacc
    import concourse.mybir as mybir
    import concourse.tile as tile

    f16 = mybir.dt.float16
    f8 = mybir.dt.float8e3
    f32 = mybir.dt.float32

    nc = bacc.Bacc("TRN2", target_bir_lowering=False, debug=False)

    xq_d = nc.dram_tensor("xqT", [P, 4, B, T], f16, kind="ExternalInput").ap()
    xkv_d = nc.dram_tensor("xkvT", [P, 4, B, TKV], f16, kind="ExternalInput").ap()
    wq_d = nc.dram_tensor("wq", [P, 4, 512], f16, kind="ExternalInput").ap()
    wk_d = nc.dram_tensor("wk", [P, 4, 512], f16, kind="ExternalInput").ap()
    wv_d = nc.dram_tensor("wv", [P, 4, 512], f16, kind="ExternalInput").ap()
    wo_d = nc.dram_tensor("wo", [P, 4, 512], f16, kind="ExternalInput").ap()
    masks_d = nc.dram_tensor("masks", [P, NQ, 2, BLK], f16, kind="ExternalInput").ap()
    ident_d = nc.dram_tensor("ident", [P, P], f16, kind="ExternalInput").ap()
    if with_qk_bias:
        bqk_d = nc.dram_tensor("bqk", [P, 8], f32, kind="ExternalInput").ap()
    if with_v_bias:
        bv_d = nc.dram_tensor("bv", [P, 512], f16, kind="ExternalInput").ap()
    if with_o_bias:
        bo_d = nc.dram_tensor("bo", [P, 512], f32, kind="ExternalInput").ap()
    out_d = nc.dram_tensor("out", [B, T, F], f16, kind="ExternalOutput").ap()

    Exp = mybir.ActivationFunctionType.Exp
    mult = mybir.AluOpType.mult

    with tile.TileContext(nc) as tc:
        with (
            tc.tile_pool(name="persist", bufs=1) as sb,
            tc.tile_pool(name="epool", bufs=3) as epool,
            tc.tile_pool(name="opool", bufs=2) as opool,
            tc.tile_pool(name="otpool", bufs=2) as otpool,
            tc.tile_pool(name="rpool", bufs=3) as rpool,
            tc.tile_pool(name="dpool", bufs=4) as dpool,
            tc.tile_pool(name="ps_s", bufs=2, space="PSUM") as ps_s_pool,
            tc.tile_pool(name="ps_av", bufs=1, space="PSUM") as ps_av_pool,
            tc.tile_pool(name="ps_big", bufs=1, space="PSUM") as ps_big_pool,
        ):
            # ---- persistent SBUF tensors ----
            wq_sb = sb.tile([P, 4, 512], f16, tag="wq")
            wk_sb = sb.tile([P, 4, 512], f16, tag="wk")
            wv_sb = sb.tile([P, 4, 512], f16, tag="wv")
            wo_sb = sb.tile([P, 4, 512], f16, tag="wo")
            masks_sb = sb.tile([P, NQ, 2, BLK], f16, tag="masks")
            id_sb = sb.tile([P, P], f16, tag="ident")
            xq_sb = sb.tile([P, 4, B, T], f16, tag="xq")
            xkv_sb = sb.tile([P, 4, B, TKV], f16, tag="xkv")
            # Q^T with explicit head-parity (par) slots: par 0 holds even heads
            # on partitions 0:64 (rest zero), par 1 holds odd heads on
            # partitions 64:128 (rest zero).  One score matmul then streams
            # both parities (N=256) against the shared kT block.  Matmuls with
            # base_partition=64 operands wedge the PE on this stack, hence the
            # zero-padding instead of 64-deep contractions.
            qTz_sb = sb.tile([P, 2, 4, B, T], f8, tag="qTz")
            kT_sb = sb.tile([P, 4, B, TKV], f8, tag="kT")
            v_sb = sb.tile([P, B, NKV, H * 65], f16, tag="v")

            # zero the pad halves before any DMA lands so they overlap the
            # input transfers (gpsimd memsets measured 3x slower: keep on DVE)
            nc.vector.memset(qTz_sb[64:128, 0], 0.0)
            nc.vector.memset(qTz_sb[0:64, 1], 0.0)
            # ones columns of V (col 64 of each head's 65-wide stripe)
            for h in range(H):
                nc.gpsimd.memset(v_sb[:, :, :, h * 65 + 64], 1.0)

            # ---- input DMAs, ordered by first use so compute starts early ----
            nc.sync.dma_start(xq_sb[:, :, 0], xq_d[:, :, 0])
            nc.sync.dma_start(wq_sb[:], wq_d[:])
            nc.sync.dma_start(xkv_sb[:, :, 0], xkv_d[:, :, 0])
            nc.sync.dma_start(wk_sb[:], wk_d[:])
            nc.sync.dma_start(wv_sb[:], wv_d[:])
            nc.sync.dma_start(xq_sb[:, :, 1], xq_d[:, :, 1])
            nc.sync.dma_start(xkv_sb[:, :, 1], xkv_d[:, :, 1])
            nc.sync.dma_start(wo_sb[:], wo_d[:])
            nc.sync.dma_start(masks_sb[:], masks_d[:])
            nc.sync.dma_start(id_sb[:], ident_d[:])
            if with_qk_bias:
                bqk_sb = sb.tile([P, 8], f32, tag="bqk")
                nc.sync.dma_start(bqk_sb[:], bqk_d[:])
            if with_v_bias:
                bv_sb = sb.tile([P, 512], f16, tag="bv")
                nc.sync.dma_start(bv_sb[:], bv_d[:])
            if with_o_bias:
                bo_sb = sb.tile([P, 512], f32, tag="bo")
                nc.sync.dma_start(bo_sb[:], bo_d[:])

            # ---- projections ----
            # psum->sbuf copy engines round-robin; psum banks alternate av/big
            _eng = [0]
            _bank = [0]

            def proj_psum():
                _bank[0] ^= 1
                if _bank[0]:
                    return ps_big_pool.tile([P, 512], f32, tag="big", name="ps_pb")
                return ps_av_pool.tile([P, 512], f32, tag="av", name="ps_pa")

            def copy_rr(dst, src):
                e = _eng[0] = (_eng[0] + 1) % 2
                if e == 0:
                    nc.scalar.copy(dst, src)
                else:
                    nc.vector.tensor_copy(dst, src)

            # QT/KT: lhsT = W tile [f, dh], rhs = xT [f, tok]
            def proj_T(w_sb, x_sb, dsts, tok_total, bias_col, b):
                per_b = tok_total // B
                for dh_t in range(4):
                    for off in range(0, per_b, 512):
                        w = min(512, per_b - off)
                        ps = proj_psum()
                        for ft in range(4):
                            nc.tensor.matmul(
                                ps[:, :w],
                                lhsT=w_sb[:, ft, dh_t * P:(dh_t + 1) * P],
                                rhs=x_sb[:, ft, b, off:off + w],
                                start=(ft == 0),
                                stop=(ft == 3),
                            )
                        for dst_sb, lo, hi_ in dsts:
                            dst = dst_sb(dh_t, b, off, w, lo, hi_)
                            src = ps[lo:hi_, :w]
                            if bias_col is not None:
                                nc.vector.tensor_scalar_add(
                                    dst, src,
                                    bqk_sb[lo:hi_, bias_col + dh_t:bias_col + dh_t + 1],
                                )
                            else:
                                copy_rr(dst, src)

            def qtz_dst(par):
                def f(dh_t, b, off, w, lo, hi_):
                    return qTz_sb[lo:hi_, par, dh_t, b, off:off + w]
                return f

            def kt_dst(dh_t, b, off, w, lo, hi_):
                return kT_sb[lo:hi_, dh_t, b, off:off + w]

            for b in range(B):
                proj_T(wq_sb, xq_sb,
                       [(qtz_dst(0), 0, 64), (qtz_dst(1), 64, 128)],
                       B * T, 0 if with_qk_bias else None, b)
                proj_T(wk_sb, xkv_sb, [(kt_dst, 0, 128)],
                       B * TKV, 4 if with_qk_bias else None, b)
                # V[tok, dh]: lhsT = xT tile [f, tok], rhs = Wv [f, dh]
                for kt in range(NKV):
                    ps = proj_psum()
                    for ft in range(4):
                        nc.tensor.matmul(
                            ps[:],
                            lhsT=xkv_sb[:, ft, b, kt * P:(kt + 1) * P],
                            rhs=wv_sb[:, ft, :],
                            start=(ft == 0),
                            stop=(ft == 3),
                        )
                    v_dst = v_sb[:, b, kt, :].rearrange("p (h x) -> p h x", h=H)[:, :, :64]
                    ps_v = ps.rearrange("p (h x) -> p h x", x=64)
                    if with_v_bias:
                        nc.vector.tensor_tensor(
                            v_dst, ps_v,
                            bv_sb.rearrange("p (h x) -> p h x", x=64), mybir.AluOpType.add,
                        )
                    else:
                        copy_rr(v_dst, ps_v)

            # ---- attention ----
            for b in range(B):
                if _STAGE == "proj":
                    break
                for qt in range(NQ):
                    o_t = opool.tile([P, 512], f16, tag="o")
                    rec_t = dpool.tile([P, H], f32, tag="rec")
                    for hg in range(2):
                        # scores: one matmul per (pr, j) streams both parities
                        ps_sc = ps_s_pool.tile([P, 6, 256], f32, tag="sc")
                        for pr in range(2):
                            dt_i = hg * 2 + pr
                            for j in range(3):
                                nc.tensor.matmul(
                                    ps_sc[:, pr * 3 + j, :],
                                    lhsT=kT_sb[:, dt_i, b, (qt + j) * P:(qt + j + 1) * P],
                                    rhs=qTz_sb[:, :, dt_i, b, qt * P:(qt + 1) * P],
                                    start=True, stop=True,
                                )
                        e_t = epool.tile([P, 6, 256], f16, tag="e")
                        nc.scalar.activation(e_t[:], ps_sc[:], Exp, scale=0.125)
                        # both side-block masks in one strided op per pr
                        # (j in {0,2}); e layout (pr, j, par, q)
                        e5 = e_t.rearrange("p (pr j) (par q) -> p pr j par q", pr=2, par=2)
                        for pr in range(2):
                            nc.vector.tensor_tensor(
                                e5[:, pr, 0::2], e5[:, pr, 0::2],
                                masks_sb[:, qt, :, None, :].to_broadcast((P, 2, 2, P)),
                                mult,
                            )
                        if _STAGE == "noav":
                            continue
                        ps_av = ps_av_pool.tile([P, 4, 128], f32, tag="av")
                        for pr in range(2):
                            for par in range(2):
                                hi = pr * 2 + par
                                h = hg * 4 + hi
                                for j in range(3):
                                    nc.tensor.matmul(
                                        ps_av[:, hi, :65],
                                        lhsT=e5[:, pr, j, par, :],
                                        rhs=v_sb[:, b, qt + j, h * 65:(h + 1) * 65],
                                        start=(j == 0), stop=(j == 2),
                                    )
                        hs = slice(hg * 4, hg * 4 + 4)
                        nc.vector.reciprocal(rec_t[:, hs], ps_av[:, :, 64])
                        nc.vector.tensor_tensor(
                            o_t[:, hg * 256:(hg + 1) * 256].rearrange("p (h x) -> p h x", x=64),
                            ps_av[:, :, 0:64],
                            rec_t[:, hs][:, :, None].to_broadcast((P, 4, 64)),
                            mult,
                        )
                    if _STAGE in ("noav", "notail"):
                        if _STAGE == "notail":
                            res_t = rpool.tile([P, 512], f16, tag="res")
                            nc.vector.tensor_copy(res_t[:], o_t[:])
                            nc.sync.dma_start(out_d[b, qt * P:(qt + 1) * P, :], res_t[:])
                        continue
                    # transpose O (regular matmuls against identity), then out proj
                    ps_ot = ps_big_pool.tile([P, 512], f32, tag="big")
                    for dt_i in range(4):
                        nc.tensor.matmul(
                            ps_ot[:, dt_i * P:(dt_i + 1) * P],
                            lhsT=o_t[:, dt_i * P:(dt_i + 1) * P],
                            rhs=id_sb[:],
                            start=True, stop=True,
                        )
                    ot_t = otpool.tile([P, 512], f16, tag="ot")
                    nc.scalar.copy(ot_t[:], ps_ot[:])
                    ps_r = ps_av_pool.tile([P, 512], f32, tag="av")
                    for dt_i in range(4):
                        nc.tensor.matmul(
                            ps_r[:],
                            lhsT=ot_t[:, dt_i * P:(dt_i + 1) * P],
                            rhs=wo_sb[:, dt_i, :],
                            start=(dt_i == 0), stop=(dt_i == 3),
                        )
                    res_t = rpool.tile([P, 512], f16, tag="res")
                    if with_o_bias:
                        nc.vector.tensor_tensor(res_t[:], ps_r[:], bo_sb[:], mybir.AluOpType.add)
                    else:
                        nc.vector.tensor_copy(res_t[:], ps_r[:])
                    nc.sync.dma_start(out_d[b, qt * P:(qt + 1) * P, :], res_t[:])

    nc.compile()
    return nc


def _part_major(a2d):
    """[K*128, N] -> [128, K, N] partition-major contiguous fp16."""
    k = a2d.shape[0] // P
    return np.ascontiguousarray(
        a2d.reshape(k, P, *a2d.shape[1:]).transpose(1, 0, *range(2, a2d.ndim + 1))
    )


def kernel(**inputs):
    inputs_q = np.asarray(inputs["inputs_q"], np.float32)
    inputs_kv = np.asarray(inputs["inputs_kv"], np.float32)
    Wq = np.asarray(inputs["Wq"], np.float32).reshape(F, H * D)
    Wk = np.asarray(inputs["Wk"], np.float32).reshape(F, H * D)
    Wv = np.asarray(inputs["Wv"], np.float32).reshape(F, H * D)
    Wo = np.asarray(inputs["Wo"], np.float32).reshape(H * D, F)
    bq = np.asarray(inputs["bq"], np.float32).reshape(H * D)
    bk = np.asarray(inputs["bk"], np.float32).reshape(H * D)
    bv = np.asarray(inputs["bv"], np.float32).reshape(H * D)
    bo = np.asarray(inputs["bo"], np.float32).reshape(F)

    with_qk_bias = bool(np.any(bq) or np.any(bk))
    with_v_bias = bool(np.any(bv))
    with_o_bias = bool(np.any(bo))

    nc = _build_program(with_qk_bias, with_v_bias, with_o_bias)

    wq_h = _part_major(Wq.astype(F16))
    wk_h = _part_major(Wk.astype(F16))
    wv_h = _part_major(Wv.astype(F16))
    wo_h = _part_major(Wo.astype(F16))
    ident = np.eye(P, dtype=F16)

    xq16 = inputs_q.astype(F16)
    xkv16 = inputs_kv.astype(F16)

    maskL = np.tril(np.ones((BLK, BLK), F16))
    maskR = np.triu(np.ones((BLK, BLK), F16), 1)
    zero = np.zeros((BLK, BLK), F16)

    in_maps = []
    for c in range(NCORES):
        t0 = c * T
        xq_c = xq16[:, t0:t0 + T, :]                      # [B, T, F]
        lo, hi = t0 - BLK, t0 + T + BLK
        kv_c = np.pad(
            xkv16[:, max(0, lo):min(S, hi), :],
            ((0, 0), (max(0, -lo), max(0, hi - S)), (0, 0)),
        )                                                  # [B, TKV, F]

        # x^T in [128, ft, b, t] layout
        xqT = np.ascontiguousarray(
            xq_c.transpose(2, 0, 1).reshape(4, P, B, T).transpose(1, 0, 2, 3)
        )
        xkvT = np.ascontiguousarray(
            kv_c.transpose(2, 0, 1).reshape(4, P, B, TKV).transpose(1, 0, 2, 3)
        )

        masks = np.empty((P, NQ, 2, BLK), F16)
        for n in range(NQ):
            g = c * NQ + n
            masks[:, n, 0] = maskL if g > 0 else zero
            masks[:, n, 1] = maskR if g < S // BLK - 1 else zero

        m = {
            "xqT": xqT, "xkvT": xkvT,
            "wq": wq_h, "wk": wk_h, "wv": wv_h, "wo": wo_h,
            "masks": masks, "ident": ident,
        }
        if with_qk_bias:
            m["bqk"] = np.ascontiguousarray(
                np.stack([bq.reshape(4, P).T, bk.reshape(4, P).T], 1).reshape(P, 8)
            )
        if with_v_bias:
            m["bv"] = np.broadcast_to(bv.astype(F16), (P, 512)).copy()
        if with_o_bias:
            m["bo"] = np.broadcast_to(bo.astype(np.float32), (P, 512)).copy()
        in_maps.append(m)

    from concourse.bass_utils import run_bass_kernel_spmd

    res = run_bass_kernel_spmd(nc, in_maps, core_ids=list(range(NCORES)))
    global LAST_RESULT
    LAST_RESULT = res
    out = np.concatenate([np.asarray(res.results[c]["out"]) for c in range(NCORES)], axis=1)
    return np.ascontiguousarray(out.astype(np.float32))


LAST_RESULT = None
